# revision 24
# baseline (speedup 1.0000x reference)
"""Spatial LocalResponseNorm (5x5 box window over H,W) on 8 TRN2 NeuronCores.

  out = x / (2.0 + 1e-4 * boxsum5x5(x^2)) ** 0.75     x: (16, 96, 224, 224) f32

Strategy (pure data parallel, batch sharded 2 per core; per core 192 images
of 224x224):

  * H is the partition axis.  Each image splits into two row tiles loaded
    with a 2-row halo: rows 0..113 (outputs rows 0..111) and rows 110..223
    (outputs rows 112..223).  K=114 for every tile.
  * The 5x5 box sum of squares is computed as 5 accumulating matmuls with a
    single stationary banded matrix B[k,m] = 1 iff |k-m| <= 2 (bf16,
    [114,128]).  Each matmul's moving operand is the squared tile shifted by
    dw in W; the band does the H-direction sum, PSUM accumulation does the
    W-direction sum.  Band clipping at k-range edges reproduces the
    reference's zero padding in H; zeroed pad columns reproduce it in W.
  * ScalarE computes d^-0.75 as Exp(-0.75 * Ln(1e-4 * ssq + 2.0)) straight
    from PSUM (both functions live in one ACT table set).
  * VectorE computes the squares (f32 -> bf16) and the final x * r multiply.

Engine budgets per core at full size all land just under the ~215us HBM
roofline (in+out 77MB @ ~360GB/s).
"""

import numpy as np
import ml_dtypes

import concourse.bass as bass
import concourse.bacc as bacc
import concourse.tile as tile
from concourse import mybir
from concourse.bass_utils import run_bass_kernel_spmd

F32 = mybir.dt.float32
BF16 = mybir.dt.bfloat16
AF = mybir.ActivationFunctionType

N_CORES = 8
H = 224
W = 224
KW = 5  # window
K_CONST = 2.0
ALPHA = 1e-4
BETA = 0.75

KROWS = 114          # input rows per tile (with halo)
ROWS_OUT = 112       # output rows per tile
IMG_PER_UNIT = 8     # images (n,c planes) processed per pipeline unit

# Banded matrix: B[k, m] = 1 iff |k - m| <= 2.  M padded to 128 so the
# stationary operand always has 128 columns (enables fast weight load).
BAND_NP = (
    np.abs(np.arange(KROWS)[:, None] - np.arange(128)[None, :]) <= (KW // 2)
).astype(ml_dtypes.bfloat16)


def _patch_act_tables():
    """Prefer the table set holding BOTH Ln and Exp so the ACT engine does a
    single table load instead of thrashing between ln/exp sets per call."""
    if getattr(bacc, "_lrn_act_tables_patched", False):
        return
    orig = bacc.get_activation_tables

    def filtered(arch):
        # Positions must stay aligned with act_info.json (position IS the
        # act_func_set_id), so instead of reordering, strip Ln/Exp from every
        # set except the combined one; the load-insertion pass then has no
        # choice but to use it for both functions.
        t = {k: set(v) for k, v in orig(arch).items()}
        combined = "natural_log_exp_and_others"
        if combined in t:
            ln_exp = {AF.Ln, AF.Exp}
            for name, fns in t.items():
                if name != combined:
                    fns -= ln_exp
        return t

    bacc.get_activation_tables = filtered
    bacc._lrn_act_tables_patched = True


def build_nc(nb: int, c: int) -> bacc.Bacc:
    """Build the per-core kernel for a shard of shape [nb, c, H, W]."""
    assert c % IMG_PER_UNIT == 0
    _patch_act_tables()
    nc = bacc.Bacc("TRN2", target_bir_lowering=False, debug=False,
                   num_devices=N_CORES)
    x_d = nc.dram_tensor("x", [nb, c, H, W], F32, kind="ExternalInput")
    band_d = nc.dram_tensor("band", [KROWS, 128], BF16, kind="ExternalInput")
    y_d = nc.dram_tensor("y", [nb, c, H, W], F32, kind="ExternalOutput")

    with tile.TileContext(nc) as tc:
        with (
            tc.tile_pool(name="const", bufs=1) as constp,
            tc.tile_pool(name="xinp", bufs=11) as xinp,
            tc.tile_pool(name="sqp", bufs=3) as sqp,
            tc.tile_pool(name="lndp", bufs=2) as lndp,
            tc.tile_pool(name="rrp", bufs=2) as rrp,
            tc.tile_pool(name="outp", bufs=10) as outp,
            tc.tile_pool(name="psump", bufs=4, space="PSUM") as psump,
        ):
            band_sb = constp.tile([KROWS, 128], BF16)
            nc.sync.dma_start(band_sb[:, :], band_d[:, :])
            bias_k = constp.tile([128, 1], F32)
            nc.vector.memset(bias_k[:, :], K_CONST)

            for n in range(nb):
                for t in range(2):
                    r0 = 0 if t == 0 else H - KROWS      # first input row
                    pv = 0 if t == 0 else 2              # valid partition base
                    rout0 = 0 if t == 0 else ROWS_OUT    # first output row
                    for ct in range(c // IMG_PER_UNIT):
                        c0 = ct * IMG_PER_UNIT

                        xin = xinp.tile([KROWS, IMG_PER_UNIT, W], F32)
                        # gpsimd dma_start goes through the software DGE,
                        # which round-robins across all 16 DMA engines;
                        # sync/scalar HWDGE queues pin to engines 64-69 and
                        # cap at ~128GB/s, so all bulk traffic stays on SWDGE.
                        # One merged 8-image DMA per direction keeps the
                        # issue rate well under the drain rate.
                        nc.gpsimd.dma_start(
                            xin[:, :, :],
                            x_d[n, c0:c0 + IMG_PER_UNIT, r0:r0 + KROWS,
                                :].rearrange("c r w -> r c w"),
                        )

                        sq = sqp.tile([KROWS, IMG_PER_UNIT, W + 4], BF16)
                        nc.vector.memset(sq[:, :, 0:2], 0.0)
                        nc.vector.memset(sq[:, :, W + 2:W + 4], 0.0)
                        nc.vector.tensor_mul(sq[:, :, 2:W + 2], xin[:, :, :],
                                             xin[:, :, :])

                        # Two 2-bank PSUM tiles per unit (4 in flight across
                        # units) keep each tile's matmul->ln lifetime short so
                        # PSUM recycling never paces the pipeline.
                        # Compute runs on partitions 0..113 (compute-engine APs
                        # must start 32-aligned); rows outside the valid range
                        # are legitimate positive partial sums, discarded at
                        # the output DMA.
                        lnd = lndp.tile([128, IMG_PER_UNIT * W], F32)
                        for h in range(2):
                            psum = psump.tile([128, 1024], F32)
                            for g2 in range(2):
                                g = 2 * h + g2
                                for dw in range(KW):
                                    nc.tensor.matmul(
                                        psum[:, g2 * 512: g2 * 512 + 2 * W],
                                        band_sb[:, :],
                                        sq[:, 2 * g: 2 * g + 2, dw: dw + W],
                                        start=(dw == 0),
                                        stop=(dw == KW - 1),
                                    )
                            psum_v = psum[0:KROWS, :].rearrange(
                                "p (g b) -> p g b", b=512)[:, :, 0:2 * W]
                            lnd_v = lnd[0:KROWS, h * 4 * W: (h + 1) * 4 * W
                                        ].rearrange("p (g b) -> p g b", b=2 * W)
                            nc.scalar.activation(lnd_v, psum_v, AF.Ln,
                                                 bias=bias_k[0:KROWS, :],
                                                 scale=ALPHA)

                        rr = rrp.tile([128, IMG_PER_UNIT * W], F32)
                        nc.scalar.activation(rr[0:KROWS, :],
                                             lnd[0:KROWS, :],
                                             AF.Exp, scale=-BETA)

                        outb = outp.tile([128, IMG_PER_UNIT, W], F32)
                        rr_v = rr[0:KROWS, :].rearrange(
                            "p (i w) -> p i w", w=W)
                        nc.vector.tensor_mul(outb[0:KROWS],
                                             xin[0:KROWS, :, :], rr_v)

                        nc.gpsimd.dma_start(
                            y_d[n, c0:c0 + IMG_PER_UNIT,
                                rout0:rout0 + ROWS_OUT, :].rearrange(
                                    "c r w -> r c w"),
                            outb[pv:pv + ROWS_OUT, :, :],
                        )
    nc.compile()
    return nc


_CACHE: dict = {}


def _get_compiled(nb: int, c: int) -> bacc.Bacc:
    key = (nb, c)
    if key not in _CACHE:
        _CACHE[key] = build_nc(nb, c)
    return _CACHE[key]


def run(x: np.ndarray, trace: bool = False, tmpdir: str | None = None):
    """Run LRN on the full input across 8 cores. Returns (y, BassKernelResults)."""
    x = np.asarray(x)
    assert x.dtype == np.float32
    n_total, c = x.shape[0], x.shape[1]
    assert n_total % N_CORES == 0
    per = n_total // N_CORES
    nc = _get_compiled(per, c)
    in_maps = [
        {"x": np.ascontiguousarray(x[i * per:(i + 1) * per]), "band": BAND_NP}
        for i in range(N_CORES)
    ]
    res = run_bass_kernel_spmd(nc, in_maps, list(range(N_CORES)), trace=trace,
                               tmpdir=tmpdir)
    y = np.concatenate([r["y"] for r in res.results], axis=0)
    return y, res


def kernel(x: np.ndarray) -> np.ndarray:
    return run(x)[0]
